# revision 4
# baseline (speedup 1.0000x reference)
"""Trainium2 Bass kernel: CausalCrossConditionalSelfAttention.

Reference computation (B=4, T=1536, C=768, H=12, D=64):
    q/k/v = x @ W{q,k,v}.T + b   -> heads [B,H,T,64]
    att   = softmax(mask(q k^T / 8))  with mask = tile(tril(512), (3,3))
    y     = att @ v  -> [B,T,C];  out = y @ Wp.T + bp

Sharding (8 cores): data-parallel over B (4) x tensor-parallel over the 12
heads in 2 groups of 6.  Each core emits a partial [1536, 768] output; the
host sums the two head-group partials per batch.  The host pre-transposes
each core's shard and converts to fp16 (layout/dtype prep for the DMA; all
FLOPs stay on device; fp16's 10 mantissa bits keep rel-err ~1e-3 against
the 2e-2 budget), pre-broadcasts the tiny bias vectors, and adds the
output bias bp during the final host-side partial-sum combine.

Per-core dataflow.  PE matmul cost on TRN2 is (output free width) x
(cycles/row); fp16 runs 1 cycle/row at ANY width (fp32r needs N>=256), so
all matmul inputs are fp16 with fp32 PSUM accumulation:
  - qT/kT = W^T.T @ xT [384,1536] head-major rows; PSUM->SBUF evacuation on
    ACT (Identity) fuses the per-partition bias, one 1536-wide instruction
    per 128-channel block.
  - v natural [1536, 6*65] with a ones column per head (AV's stationary
    [tk, 65] then also produces the softmax denominator row for free).
  - Attention per (q-block qb of 512, head h), exploiting that the 3x3
    tiled mask repeats the same tril(512) in every block: for k-tile m
    (128 rows), only q >= 128*m is computed (exact triangle; the three
    512-blocks a=0,1,2 share the pattern).  The 3 a-tiles of one m land in
    the 3 PSUM banks of one [128,1536] group tile -- m2 and m3 pack into
    one bank (256+128 cols) -- so exp(S/8) is 3 ACT instructions per
    (qb,h), strided across banks, writing fp16 es tiles.  The tril mask
    multiplies only the 128-wide diagonal sub-tiles (strided fp16 ops on
    the otherwise-idle Pool engine).
  - AV accumulates 12 [tk,65]x[tk,q] matmuls into av [65,512]; row 64 is
    the denominator l.  recip(l) on DVE, broadcast to 64 partitions on
    Pool, normalize during the PSUM->SBUF evacuation (ordered last on DVE
    each round so the in-order DVE never blocks on the broadcast).
  - Engine balance: ACT exp (3840 cols * 0.83ns) slightly exceeds PE's S+AV
    (7680 cols * 0.42ns) per iteration, so v-projection and the per-qb
    output projections are interleaved INTO the attention pipeline (AV
    bursts lag S bursts by AVLAG=5 iterations) to keep PE the only busy
    engine.
  - DMAs are batched (one per weight tensor, 2 x-chunks, one fp32 const
    blob, paired output tiles): each dma_start costs ~650ns of serialized
    HWDGE time regardless of size.
"""

import math
from contextlib import ExitStack, nullcontext

import numpy as np

import concourse.bass as bass
import concourse.bacc as bacc
import concourse.mybir as mybir
import concourse.tile as tile
from concourse.bass_utils import run_bass_kernel_spmd

F32 = mybir.dt.float32
F16 = mybir.dt.float16
AF = mybir.ActivationFunctionType

B, T, C = 4, 1536, 768
H = 12
D = 64
NCORES = 8
HG = H // 2       # heads per core (6)
CL = HG * D       # local channels per core (384)
VW = D + 1        # v tile width per head incl. ones column (65)
NC32 = 3 + 3 + CL     # fp32 const blob cols (bq, bk, bv_bc); bp is host-added
AVLAG = 5         # AV burst lags S burst by this many iterations


def build_nc(loop_reps=None):
    # loop_reps: when set, wrap the whole body in a hardware For loop that
    # re-executes it loop_reps times per dispatch.  Used only by test.py's
    # timing harness (amortizes host dispatch overhead); the graded
    # kernel() path always builds the single-shot variant.
    nc = bacc.Bacc("TRN2", target_bir_lowering=False, debug=False,
                   enable_asserts=False)

    xt_d = nc.dram_tensor("xt", [C, T], F16, kind="ExternalInput").ap()
    wqt_d = nc.dram_tensor("wqt", [C, CL], F16, kind="ExternalInput").ap()
    wkt_d = nc.dram_tensor("wkt", [C, CL], F16, kind="ExternalInput").ap()
    wvt_d = nc.dram_tensor("wvt", [C, CL], F16, kind="ExternalInput").ap()
    wpt_d = nc.dram_tensor("wpt", [CL, C], F16, kind="ExternalInput").ap()
    cst32_d = nc.dram_tensor("cst32", [128, NC32], F32, kind="ExternalInput").ap()
    mask6_d = nc.dram_tensor("mask6", [128, 768], F16, kind="ExternalInput").ap()
    out_d = nc.dram_tensor("out", [T, C], F32, kind="ExternalOutput").ap()

    scale = 1.0 / math.sqrt(D)

    with tile.TileContext(nc) as tc, ExitStack() as ctx, \
            (tc.For_i(0, loop_reps, 1) if loop_reps else nullcontext()):
        constp = ctx.enter_context(tc.tile_pool(name="constp", bufs=1))
        wtp = ctx.enter_context(tc.tile_pool(name="wtp", bufs=1))
        qkvp = ctx.enter_context(tc.tile_pool(name="qkvp", bufs=1))
        iop = ctx.enter_context(tc.tile_pool(name="iop", bufs=2))
        # PSUM: 2 x [128,1536]f32 3-bank group tiles + 2 x [65,512] AV banks
        psp = ctx.enter_context(tc.tile_pool(name="psp", bufs=2, space="PSUM"))
        avp = ctx.enter_context(tc.tile_pool(name="avp", bufs=2, space="PSUM"))
        attp = ctx.enter_context(tc.tile_pool(name="attp", bufs=1))

        def sgroup():
            return psp.tile([128, 1536], F32, tag="sg", name="sg")

        # ---- ACT warmup: load the activation table before it's needed ----
        scr = constp.tile([1, 8], F32)
        nc.vector.memset(scr[:], 0.0)
        nc.scalar.activation(scr[:], scr[:], AF.Exp, scale=1.0)

        # ---- DMAs, ordered so q-proj unblocks first ----
        xa = wtp.tile([128, 6 * T], F16, name="xa")
        xv = xa.rearrange("p (a t) -> p a t", t=T)
        xd = xt_d.rearrange("(a p) t -> p a t", p=128)
        wq = wtp.tile([128, 6 * CL], F16, name="wq")
        wqv = wq.rearrange("p (a w) -> p a w", w=CL)
        wqd = wqt_d.rearrange("(a p) w -> p a w", p=128)
        cst32 = constp.tile([128, NC32], F32)
        # first two transfers are the minimal operand set of the very first
        # matmul (x cols 0:256 + wq's kc0 slice): everything else queues
        # behind them on the serialized DMA pipe
        nc.sync.dma_start(xv[:, :, 0:256], xd[:, :, 0:256])
        nc.sync.dma_start(wqv[:, 0:1], wqd[:, 0:1])
        nc.sync.dma_start(wqv[:, 1:3], wqd[:, 1:3])
        # bq/bk columns early (tiny): the first q evacuation needs them
        nc.sync.dma_start(cst32[:, 0:6], cst32_d[:, 0:6])
        nc.sync.dma_start(xv[:, :, 256:512], xd[:, :, 256:512])
        nc.sync.dma_start(wqv[:, 3:6], wqd[:, 3:6])
        nc.sync.dma_start(xv[:, :, 512:1024], xd[:, :, 512:1024])
        nc.sync.dma_start(xv[:, :, 1024:T], xd[:, :, 1024:T])
        nc.sync.dma_start(cst32[:, 6:NC32], cst32_d[:, 6:NC32])
        bq_sb = cst32[:, 0:3]
        bk_sb = cst32[:, 3:6]
        bv_bc = cst32[:, 6:6 + CL]
        wk = wtp.tile([128, 6 * CL], F16, name="wk")
        nc.sync.dma_start(
            wk.rearrange("p (a w) -> p a w", w=CL),
            wkt_d.rearrange("(a p) w -> p a w", p=128))
        wv = wtp.tile([128, 6 * CL], F16, name="wv")
        nc.sync.dma_start(
            wv.rearrange("p (a w) -> p a w", w=CL),
            wvt_d.rearrange("(a p) w -> p a w", p=128))
        mask6 = constp.tile([128, 768], F16)
        nc.sync.dma_start(mask6[:], mask6_d[:])
        wp = wtp.tile([128, 3 * C], F16, name="wp")
        nc.sync.dma_start(
            wp.rearrange("p (a w) -> p a w", w=C),
            wpt_d.rearrange("(a p) w -> p a w", p=128))

        def xT(kc):
            return xa[:, kc * T:(kc + 1) * T]

        def wview(w, kc):
            return w[:, kc * CL:(kc + 1) * CL]

        # ---- q/k projections ----
        # qT/kT [384, 1536] as 3 tiles [128, 1536] (partition = out-channel);
        # evacuation+bias on ACT, one instruction per (proj, mo)
        qT = [qkvp.tile([128, T], F16, name=f"qT{i}") for i in range(3)]
        kT = [qkvp.tile([128, T], F16, name=f"kT{i}") for i in range(3)]
        def emit_proj_group(dst, w, bias, mo, on_act, chunks):
            ps = sgroup()
            for no, wd_ in chunks:
                for kc in range(6):
                    nc.tensor.matmul(
                        ps[:, no:no + wd_],
                        wview(w, kc)[:, mo * 128:(mo + 1) * 128],
                        xT(kc)[:, no:no + wd_],
                        start=(kc == 0), stop=(kc == 5))
            if on_act:
                nc.scalar.activation(dst[mo][:], ps[:], AF.Identity,
                                     bias=bias[:, mo:mo + 1], scale=1.0)
            else:
                # k evacs split across ACT/DVE (GPSIMD cannot touch PSUM) so
                # the S-group buffer frees fast right before attention starts
                b = bias[:, mo:mo + 1]
                nc.scalar.activation(dst[mo][:, 0:1024], ps[:, 0:1024],
                                     AF.Identity, bias=b, scale=1.0)
                nc.vector.tensor_scalar_add(dst[mo][:, 1024:T],
                                            ps[:, 1024:T], b)

        # q/k projections; q nt0 split 2x256 so PE starts on the first x chunk
        for mo in range(3):
            chunks = ((0, 256), (256, 256), (512, 512), (1024, 512)) \
                if mo == 0 else ((0, 512), (512, 512), (1024, 512))
            emit_proj_group(qT, wq, bq_sb, mo, True, chunks)
        # ---- v projection pieces (interleaved into attention below) ----
        # v natural [1536, 6*65] with ones col per head; 12 tiles [128, 390]
        vaug = [qkvp.tile([128, HG * VW], F16, name=f"vaug{mt}")
                for mt in range(12)]
        for mt in range(12):
            vones = vaug[mt].rearrange("p (h w) -> p h w", w=VW)[:, :, D:VW]
            nc.vector.memset(vones, 1.0)

        def emit_v_group(g):  # 3 token-tiles per PSUM group
            ps = sgroup()
            for j in range(3):
                mt = g * 3 + j
                for kc in range(6):
                    nc.tensor.matmul(ps[:, j * 512:j * 512 + CL],
                                     xT(kc)[:, mt * 128:(mt + 1) * 128],
                                     wview(wv, kc)[:], start=(kc == 0),
                                     stop=(kc == 5))
            for j in range(3):
                mt = g * 3 + j
                vdst = vaug[mt].rearrange("p (h w) -> p h w", w=VW)[:, :, 0:D]
                nc.vector.tensor_add(
                    vdst,
                    ps[:, j * 512:j * 512 + CL].rearrange(
                        "p (h w) -> p h w", w=D),
                    bv_bc.rearrange("p (h w) -> p h w", w=D))

        # mo descending: the last k evac (mo0) is exactly what the first
        # S-burst needs, so the pool-claim wait and the data wait coincide.
        # vg0 sits between k:mo1 and k:mo0 so the first S-burst's second
        # pool claim also waits only on already-finished work.
        emit_proj_group(kT, wk, bk_sb, 2, False,
                        ((0, 512), (512, 512), (1024, 512)))
        emit_proj_group(kT, wk, bk_sb, 1, False,
                        ((0, 512), (512, 512), (1024, 512)))
        emit_v_group(0)
        emit_proj_group(kT, wk, bk_sb, 0, False,
                        ((0, 512), (512, 512), (1024, 512)))
        emit_v_group(1)

        # ---- attention ----
        yT = [attp.tile([128, T], F16, name=f"yT{i}") for i in range(3)]

        def emit_s_burst(qb, h):
            kti, koff = h // 2, (h % 2) * D

            def smm(sg_dst, m, wd):
                for a in range(3):
                    nc.tensor.matmul(
                        sg_dst(a),
                        kT[kti][koff:koff + D,
                                a * 512 + m * 128:a * 512 + m * 128 + 128],
                        qT[kti][koff:koff + D,
                                qb * 512 + m * 128:(qb + 1) * 512],
                        start=True, stop=True)

            es = {}
            for m, wd in ((0, 512), (1, 384)):
                sg = sgroup()
                smm(lambda a, m=m, wd=wd: sg[:, a * 512:a * 512 + wd], m, wd)
                e = attp.tile([128, 3 * wd], F16, tag=f"esm{m}", bufs=AVLAG + 1,
                              name="es")
                nc.scalar.activation(
                    e.rearrange("p (a w) -> p a w", w=wd),
                    sg.rearrange("p (a c) -> p a c", c=512)[:, :, 0:wd],
                    AF.Exp, scale=scale)
                # tril mask on the three diagonal 128-wide sub-tiles
                # (on the idle Pool engine; the AV lag gives ample slack)
                dv = e.rearrange("p (a w) -> p a w", w=wd)[:, :, 0:128]
                nc.gpsimd.tensor_mul(
                    dv, dv,
                    mask6[:, 0:384].rearrange("p (a w) -> p a w", w=128))
                es[m] = e
            # m2 (256) and m3 (128) pack into one bank: one exp, one mask
            sg = sgroup()
            smm(lambda a: sg[:, a * 512:a * 512 + 256], 2, 256)
            smm(lambda a: sg[:, a * 512 + 256:a * 512 + 384], 3, 128)
            e23 = attp.tile([128, 3 * 384], F16, tag="esm23", bufs=AVLAG + 1,
                            name="es")
            nc.scalar.activation(
                e23.rearrange("p (a w) -> p a w", w=384),
                sg.rearrange("p (a c) -> p a c", c=512)[:, :, 0:384],
                AF.Exp, scale=scale)
            # diag sub-tiles: m2's at slab offset 0, m3's at offset 256
            dv = e23.rearrange("p (a j w) -> p a j w", j=3, w=128)[:, :, 0:3:2, :]
            nc.gpsimd.tensor_mul(
                dv, dv,
                mask6.rearrange("p (a j w) -> p a j w", j=2, w=128))
            es[23] = e23
            return es

        def emit_av_recl(qb, h, es, use_sg=False):
            kti, koff = h // 2, (h % 2) * D
            # drain iterations borrow a freed S-group bank so the
            # recl->bcast->normalize chain of consecutive AVs overlaps
            av = (sgroup()[0:VW, 0:512] if use_sg
                  else avp.tile([VW, 512], F32, tag="av", name="av_ps"))
            mv = {0: lambda a: es[0][:, a * 512:(a + 1) * 512],
                  1: lambda a: es[1][:, a * 384:(a + 1) * 384],
                  2: lambda a: es[23][:, a * 384:a * 384 + 256],
                  3: lambda a: es[23][:, a * 384 + 256:a * 384 + 384]}
            first = True
            for m in range(4):
                for a in range(3):
                    nc.tensor.matmul(
                        av[:, m * 128:512],
                        vaug[a * 4 + m][:, h * VW:(h + 1) * VW],
                        mv[m](a),
                        start=first, stop=(m == 3 and a == 2))
                    first = False
            recl = attp.tile([1, 512], F32, tag="recl", bufs=2, name="recl")
            nc.vector.reciprocal(recl[:], av[D:D + 1, :])
            return av, recl

        def emit_bcast(recl):
            bc_sb = attp.tile([D, 512], F32, tag="bc_sb", bufs=2, name="bc_sb")
            nc.gpsimd.partition_broadcast(bc_sb[:], recl[:], channels=D)
            return bc_sb

        def emit_norm(qb, h, av, bc_sb):
            kti, koff = h // 2, (h % 2) * D
            nc.vector.tensor_mul(
                yT[kti][koff:koff + D, qb * 512:(qb + 1) * 512],
                av[0:D, :], bc_sb[:])

        # Final q-block: per tile-pair, the kc0/kc1 partial accumulations
        # run first (they need only heads 0-3's normalizes, long done) so
        # PE covers the recl->bcast->norm(17) chain; only the closing kc2
        # matmuls wait on the last normalize.  Both tiles' groups stay open
        # (2 claims, one bank per chunk).  Pair (8,9)'s partials are
        # pre-staged two drain rounds early, hidden under the AV bursts.
        qb2_pre = {}

        def emit_out_qb2_pre(tpair):
            for mt in tpair:
                ps = sgroup()
                qb2_pre[mt] = ps
                for no, w in ((0, 512), (512, 256)):
                    for kc in (0, 1):
                        nc.tensor.matmul(
                            ps[:, no:no + w],
                            yT[kc][:, mt * 128:(mt + 1) * 128],
                            wp[:, kc * C + no:kc * C + no + w],
                            start=(kc == 0), stop=False)

        def emit_out_proj_qb2():
            for tpair in ((8, 9), (10, 11)):
                osb = iop.tile([128, 2 * C], F32, tag="osb", name="osb")
                if tpair[0] not in qb2_pre:
                    emit_out_qb2_pre(tpair)
                pss = qb2_pre
                for j, mt in enumerate(tpair):
                    ps = pss[mt]
                    for no, w in ((0, 512), (512, 256)):
                        nc.tensor.matmul(
                            ps[:, no:no + w],
                            yT[2][:, mt * 128:(mt + 1) * 128],
                            wp[:, 2 * C + no:2 * C + no + w],
                            start=False, stop=True)
                    if mt == 11:
                        # last tile: split evac+DMA at the chunk boundary
                        for ho, hw_ in ((0, 512), (512, 256)):
                            nc.vector.tensor_copy(
                                osb[:, j * C + ho:j * C + ho + hw_],
                                ps[:, ho:ho + hw_])
                            nc.sync.dma_start(
                                out_d[mt * 128:(mt + 1) * 128, ho:ho + hw_],
                                osb[:, j * C + ho:j * C + ho + hw_])
                    else:
                        nc.vector.tensor_copy(osb[:, j * C:(j + 1) * C],
                                               ps[:, 0:C])
                        nc.sync.dma_start(out_d[mt * 128:(mt + 1) * 128, :],
                                          osb[:, j * C:(j + 1) * C])

        def emit_out_proj(qb):
            if qb == 2:
                emit_out_proj_qb2()
                return
            singles = False
            for pair in range(2):
                osb = iop.tile([128, 2 * C], F32, tag="osb", name="osb")
                for j in range(2):
                    mt = qb * 4 + pair * 2 + j
                    ps = sgroup()
                    for no, w in ((0, 512), (512, 256)):
                        for kc in range(3):
                            nc.tensor.matmul(
                                ps[:, no:no + w],
                                yT[kc][:, mt * 128:(mt + 1) * 128],
                                wp[:, kc * C + no:kc * C + no + w],
                                start=(kc == 0), stop=(kc == 2))
                    nc.vector.tensor_copy(osb[:, j * C:(j + 1) * C],
                                          ps[:, 0:C])
                if not singles:
                    mt0 = qb * 4 + pair * 2
                    nc.sync.dma_start(
                        out_d[mt0 * 128:(mt0 + 2) * 128, :].rearrange(
                            "(j p) c -> p j c", p=128),
                        osb.rearrange("p (j c) -> p j c", c=C))

        iters = [(qb, h) for qb in range(3) for h in range(HG)]
        es_q = [None] * len(iters)
        # out-proj for qb emitted once AV index reaches (last iter of qb)+3,
        # giving the DVE normalize ample slack
        pending = []  # (ready_av_index, qb)

        # Round structure (one AV round j paired with one S round i):
        # AV(j)+recl(j) | S(i) exps/masks | bcast(j) | out-proj | norm(j).
        # The normalize goes LAST in DVE program order so the in-order DVE
        # never blocks on the Pool broadcast with masks/evacs queued behind.
        def emit_round(j, i):
            acc = None
            if j is not None:
                acc = emit_av_recl(*iters[j], es_q[j],
                                   use_sg=False)
                es_q[j] = None
            if i is not None and i < 2:
                emit_v_group(i + 2)
            if i is not None:
                es_q[i] = emit_s_burst(*iters[i])
            if j is not None:
                bc = emit_bcast(acc[1])
                if iters[j][1] == HG - 1:
                    pending.append((j + 2, iters[j][0]))
                while pending and pending[0][0] <= j:
                    emit_out_proj(pending.pop(0)[1])
                emit_norm(*iters[j], acc[0], bc)

        for i in range(len(iters)):
            emit_round(i - AVLAG if i >= AVLAG else None, i)
        for j in range(len(iters) - AVLAG, len(iters)):
            emit_round(j, None)
        while pending:
            emit_out_proj(pending.pop(0)[1])

    nc.compile()
    return nc


_NC_CACHE = None


def _get_nc():
    global _NC_CACHE
    if _NC_CACHE is None:
        _NC_CACHE = build_nc()
    return _NC_CACHE


def make_in_maps(inputs):
    x = np.asarray(inputs["x"], dtype=np.float32)
    wq = np.asarray(inputs["Wq"], np.float32)
    wk = np.asarray(inputs["Wk"], np.float32)
    wv = np.asarray(inputs["Wv"], np.float32)
    wp = np.asarray(inputs["Wp"], np.float32)
    bq = np.asarray(inputs["bq"], np.float32)
    bk = np.asarray(inputs["bk"], np.float32)
    bv = np.asarray(inputs["bv"], np.float32)
    bp = np.asarray(inputs["bp"], np.float32)
    triu = np.triu(np.ones((128, 128), dtype=np.float16))
    mask6 = np.tile(triu, (1, 6))
    in_maps = []
    for c in range(NCORES):
        b, g = c // 2, c % 2
        sl = slice(g * CL, (g + 1) * CL)
        cst32 = np.concatenate([
            bq[sl].reshape(3, 128).T,
            bk[sl].reshape(3, 128).T,
            np.tile(bv[sl][None, :], (128, 1)),
        ], axis=1).astype(np.float32)
        m = {
            "xt": np.ascontiguousarray(x[b].T.astype(np.float16)),
            "wqt": np.ascontiguousarray(wq[sl].T.astype(np.float16)),
            "wkt": np.ascontiguousarray(wk[sl].T.astype(np.float16)),
            "wvt": np.ascontiguousarray(wv[sl].T.astype(np.float16)),
            "wpt": np.ascontiguousarray(wp[:, sl].T.astype(np.float16)),
            "cst32": np.ascontiguousarray(cst32),
            "mask6": mask6,
        }
        in_maps.append(m)
    return in_maps


def combine_outputs(results, bp):
    # the output bias is added here: the host already sums the two
    # head-group partials, so folding bp in costs nothing on device
    out = np.empty((B, T, C), dtype=np.float32)
    for b in range(B):
        out[b] = results[2 * b]["out"] + results[2 * b + 1]["out"] + bp
    return out


def kernel(**inputs):
    nc = _get_nc()
    res = run_bass_kernel_spmd(nc, make_in_maps(inputs),
                               core_ids=list(range(NCORES)))
    return combine_outputs(res.results,
                           np.asarray(inputs["bp"], np.float32))



# revision 16
# speedup vs baseline: 3.5970x; 3.5970x over previous
"""Trainium2 Bass kernel: CausalCrossConditionalSelfAttention.

Reference computation (B=4, T=1536, C=768, H=12, D=64):
    q/k/v = x @ W{q,k,v}.T + b   -> heads [B,H,T,64]
    att   = softmax(mask(q k^T / 8))  with mask = tile(tril(512), (3,3))
    y     = att @ v  -> [B,T,C];  out = y @ Wp.T + bp

Sharding (8 cores): data-parallel over B (4) x tensor-parallel over the 12
heads in 2 groups of 6.  Each core emits a partial [1536, 768] output; the
host sums the two head-group partials per batch.  The host pre-transposes
each core's shard and converts to fp16 (layout/dtype prep for the DMA; all
FLOPs stay on device; fp16's 10 mantissa bits keep rel-err ~1e-3 against
the 2e-2 budget), pre-broadcasts the tiny bias vectors, and adds the
output bias bp during the final host-side partial-sum combine.

Per-core dataflow.  PE matmul cost on TRN2 is (output free width) x
(cycles/row); fp16 runs 1 cycle/row at ANY width (fp32r needs N>=256), so
all matmul inputs are fp16 with fp32 PSUM accumulation:
  - qT/kT = W^T.T @ xT [384,1536] head-major rows; PSUM->SBUF evacuation on
    ACT (Identity) fuses the per-partition bias, one 1536-wide instruction
    per 128-channel block.
  - v natural [1536, 6*128] with 64 ones columns per head: AV's
    stationary [tk, 128] then lands y in PSUM rows 0:64 and the softmax
    denominator l replicated across rows 64:128 for free.
  - Attention per (q-block qb of 512, head h), exploiting that the 3x3
    tiled mask repeats the same tril(512) in every block: for k-tile m
    (128 rows), only q >= 128*m is computed (exact triangle; the three
    512-blocks a=0,1,2 share the pattern).  The 3 a-tiles of one m land in
    the 3 PSUM banks of one [128,1536] group tile -- m2 and m3 pack into
    one bank (256+128 cols) -- so exp(S/8) is 3 ACT instructions per
    (qb,h), strided across banks, writing fp16 es tiles.  The tril mask
    multiplies only the 128-wide diagonal sub-tiles (DVE, fp16 2x modes).
  - AV accumulates 12 [tk,128]x[tk,q] matmuls into a [128,512] PSUM
    bank; recip of rows 64:128 on DVE gives a [64,512] SBUF reciprocal
    directly, and the normalize multiplies av[0:64] by it during the
    PSUM->SBUF evacuation on DVE -- no cross-partition broadcast.  The
    GPSIMD engine is left COMPLETELY IDLE: its software-implemented ops
    (partition_broadcast, tensor ops) carry large and badly-modeled
    overheads on real hardware.
  - Engine balance: ACT exp (~3.7us/iter incl per-instr init) slightly
    exceeds PE's S+AV (~3.4us), so v-projection and the per-qb output
    projections are interleaved INTO the attention pipeline (AV bursts lag
    S bursts by AVLAG=5 iterations) to keep PE the only busy engine.
  - DMAs are batched (one per weight tensor, x in 4 chunks, one fp32 const
    blob, paired output tiles): each dma_start costs ~650ns of serialized
    HWDGE time regardless of size.
"""

import math
from contextlib import ExitStack, nullcontext

import numpy as np

import concourse.bass as bass
import concourse.bacc as bacc
import concourse.mybir as mybir
import concourse.tile as tile
from concourse.bass_utils import run_bass_kernel_spmd

F32 = mybir.dt.float32
F16 = mybir.dt.float16
AF = mybir.ActivationFunctionType

B, T, C = 4, 1536, 768
H = 12
D = 64
NCORES = 8
HG = H // 2       # heads per core (6)
CL = HG * D       # local channels per core (384)
NC32 = 3 + 3 + CL     # fp32 const blob cols (bq, bk, bv_bc); bp is host-added
AVLAG = 5         # AV burst lags S burst by this many iterations


def build_nc(loop_reps=None):
    # loop_reps: when set, wrap the whole body in a hardware For loop that
    # re-executes it loop_reps times per dispatch.  Used only by test.py's
    # timing harness (amortizes host dispatch overhead); the graded
    # kernel() path always builds the single-shot variant.
    nc = bacc.Bacc("TRN2", target_bir_lowering=False, debug=False,
                   enable_asserts=False)

    xt_d = nc.dram_tensor("xt", [C, T], F16, kind="ExternalInput").ap()
    wqt_d = nc.dram_tensor("wqt", [C, CL], F16, kind="ExternalInput").ap()
    wkt_d = nc.dram_tensor("wkt", [C, CL], F16, kind="ExternalInput").ap()
    wvt_d = nc.dram_tensor("wvt", [C, CL], F16, kind="ExternalInput").ap()
    wpt_d = nc.dram_tensor("wpt", [CL, C], F16, kind="ExternalInput").ap()
    cst32_d = nc.dram_tensor("cst32", [128, NC32], F32, kind="ExternalInput").ap()
    mask6_d = nc.dram_tensor("mask6", [128, 768], F16, kind="ExternalInput").ap()
    out_d = nc.dram_tensor("out", [T, C], F32, kind="ExternalOutput").ap()

    scale = 1.0 / math.sqrt(D)

    with tile.TileContext(nc) as tc, ExitStack() as ctx, \
            (tc.For_i(0, loop_reps, 1) if loop_reps else nullcontext()):
        constp = ctx.enter_context(tc.tile_pool(name="constp", bufs=1))
        wtp = ctx.enter_context(tc.tile_pool(name="wtp", bufs=1))
        qkvp = ctx.enter_context(tc.tile_pool(name="qkvp", bufs=1))
        iop = ctx.enter_context(tc.tile_pool(name="iop", bufs=2))
        # PSUM: 2 x [128,1536]f32 3-bank group tiles + 2 x [128,512] AV banks
        psp = ctx.enter_context(tc.tile_pool(name="psp", bufs=2, space="PSUM"))
        avp = ctx.enter_context(tc.tile_pool(name="avp", bufs=2, space="PSUM"))
        attp = ctx.enter_context(tc.tile_pool(name="attp", bufs=1))

        def sgroup():
            return psp.tile([128, 1536], F32, tag="sg", name="sg")

        # ---- ACT warmup: load the activation table before it's needed ----
        scr = constp.tile([1, 8], F32)
        nc.vector.memset(scr[:], 0.0)
        nc.scalar.activation(scr[:], scr[:], AF.Exp, scale=1.0)

        # ---- DMAs, ordered so q-proj unblocks first ----
        xa = wtp.tile([128, 6 * T], F16, name="xa")
        xv = xa.rearrange("p (a t) -> p a t", t=T)
        xd = xt_d.rearrange("(a p) t -> p a t", p=128)
        wq = wtp.tile([128, 6 * CL], F16, name="wq")
        wqv = wq.rearrange("p (a w) -> p a w", w=CL)
        wqd = wqt_d.rearrange("(a p) w -> p a w", p=128)
        cst32 = constp.tile([128, NC32], F32)
        # first two transfers are the minimal operand set of the very first
        # matmul (x cols 0:256 + wq's kc0 slice): everything else queues
        # behind them on the serialized DMA pipe
        nc.sync.dma_start(wqv[:, 0:1], wqd[:, 0:1])
        nc.sync.dma_start(xv[:, :, 0:256], xd[:, :, 0:256])
        nc.sync.dma_start(wqv[:, 1:3], wqd[:, 1:3])
        # bq/bk columns early (tiny): the first q evacuation needs them
        nc.sync.dma_start(cst32[:, 0:6], cst32_d[:, 0:6])
        nc.sync.dma_start(xv[:, :, 256:512], xd[:, :, 256:512])
        nc.sync.dma_start(wqv[:, 3:6], wqd[:, 3:6])
        nc.sync.dma_start(xv[:, :, 512:1024], xd[:, :, 512:1024])
        nc.sync.dma_start(xv[:, :, 1024:T], xd[:, :, 1024:T])
        nc.sync.dma_start(cst32[:, 6:NC32], cst32_d[:, 6:NC32])
        bq_sb = cst32[:, 0:3]
        bk_sb = cst32[:, 3:6]
        bv_bc = cst32[:, 6:6 + CL]
        wk = wtp.tile([128, 6 * CL], F16, name="wk")
        nc.sync.dma_start(
            wk.rearrange("p (a w) -> p a w", w=CL),
            wkt_d.rearrange("(a p) w -> p a w", p=128))
        wv = wtp.tile([128, 6 * CL], F16, name="wv")
        nc.sync.dma_start(
            wv.rearrange("p (a w) -> p a w", w=CL),
            wvt_d.rearrange("(a p) w -> p a w", p=128))
        mask6 = constp.tile([128, 768], F16)
        nc.sync.dma_start(mask6[:], mask6_d[:])
        wp = wtp.tile([128, 3 * C], F16, name="wp")
        nc.sync.dma_start(
            wp.rearrange("p (a w) -> p a w", w=C),
            wpt_d.rearrange("(a p) w -> p a w", p=128))

        def xT(kc):
            return xa[:, kc * T:(kc + 1) * T]

        def wview(w, kc):
            return w[:, kc * CL:(kc + 1) * CL]

        # ---- q/k projections ----
        # qT/kT [384, 1536] as 3 tiles [128, 1536] (partition = out-channel);
        # evacuation+bias on ACT, one instruction per (proj, mo)
        qT = [qkvp.tile([128, T], F16, name=f"qT{i}") for i in range(3)]
        kT = [qkvp.tile([128, T], F16, name=f"kT{i}") for i in range(3)]
        def emit_proj_group(dst, w, bias, mo, on_act, chunks):
            ps = sgroup()
            for no, wd_ in chunks:
                for kc in range(6):
                    nc.tensor.matmul(
                        ps[:, no:no + wd_],
                        wview(w, kc)[:, mo * 128:(mo + 1) * 128],
                        xT(kc)[:, no:no + wd_],
                        start=(kc == 0), stop=(kc == 5))
            if on_act:
                nc.scalar.activation(dst[mo][:], ps[:], AF.Identity,
                                     bias=bias[:, mo:mo + 1], scale=1.0)
            else:
                # k evacs split across ACT/DVE so the S-group buffer frees
                # fast right before attention starts
                b = bias[:, mo:mo + 1]
                nc.scalar.activation(dst[mo][:, 0:1024], ps[:, 0:1024],
                                     AF.Identity, bias=b, scale=1.0)
                nc.vector.tensor_scalar_add(dst[mo][:, 1024:T],
                                            ps[:, 1024:T], b)

        # q/k projections; q nt0 split 2x256 so PE starts on the first x chunk
        for mo in range(3):
            chunks = ((0, 256), (256, 256), (512, 512), (1024, 512)) \
                if mo == 0 else ((0, 512), (512, 512), (1024, 512))
            emit_proj_group(qT, wq, bq_sb, mo, True, chunks)
        # ---- v projection pieces (interleaved into attention below) ----
        # v natural [1536, 6*128] with SIXTY-FOUR ones columns per head: the
        # AV matmul then lands y in PSUM rows 0:64 and the softmax
        # denominator l replicated across rows 64:128 of the same bank, so
        # the reciprocal is a single [64,512] DVE op straight to SBUF and
        # the normalize needs no cross-partition broadcast at all (PSUM x
        # SBUF operands -- hardware allows only one PSUM input).
        vaug = [qkvp.tile([128, HG * 2 * D], F16, name=f"vaug{mt}")
                for mt in range(12)]
        for mt in range(12):
            vones = vaug[mt].rearrange("p (h w) -> p h w", w=2 * D)[:, :, D:]
            nc.vector.memset(vones, 1.0)

        def emit_v_group(g):  # 3 token-tiles per PSUM group
            ps = sgroup()
            for j in range(3):
                mt = g * 3 + j
                for kc in range(6):
                    nc.tensor.matmul(ps[:, j * 512:j * 512 + CL],
                                     xT(kc)[:, mt * 128:(mt + 1) * 128],
                                     wview(wv, kc)[:], start=(kc == 0),
                                     stop=(kc == 5))
            for j in range(3):
                mt = g * 3 + j
                vdst = vaug[mt].rearrange("p (h w) -> p h w",
                                          w=2 * D)[:, :, 0:D]
                nc.vector.tensor_add(
                    vdst,
                    ps[:, j * 512:j * 512 + CL].rearrange(
                        "p (h w) -> p h w", w=D),
                    bv_bc.rearrange("p (h w) -> p h w", w=D))

        # mo descending: the last k evac (mo0) is exactly what the first
        # S-burst needs, so the pool-claim wait and the data wait coincide.
        # vg0 sits between k:mo1 and k:mo0 so the first S-burst's second
        # pool claim also waits only on already-finished work.
        emit_proj_group(kT, wk, bk_sb, 2, False,
                        ((0, 512), (512, 512), (1024, 512)))
        emit_proj_group(kT, wk, bk_sb, 1, False,
                        ((0, 512), (512, 512), (1024, 512)))
        emit_v_group(0)
        emit_proj_group(kT, wk, bk_sb, 0, False,
                        ((0, 512), (512, 512), (1024, 512)))
        emit_v_group(1)

        # ---- attention ----
        yT = [attp.tile([128, T], F16, name=f"yT{i}") for i in range(3)]

        def emit_s_burst(qb, h):
            kti, koff = h // 2, (h % 2) * D

            def smm(sg_dst, m, wd):
                for a in range(3):
                    nc.tensor.matmul(
                        sg_dst(a),
                        kT[kti][koff:koff + D,
                                a * 512 + m * 128:a * 512 + m * 128 + 128],
                        qT[kti][koff:koff + D,
                                qb * 512 + m * 128:(qb + 1) * 512],
                        start=True, stop=True)

            es = {}
            for m, wd in ((0, 512), (1, 384)):
                sg = sgroup()
                smm(lambda a, m=m, wd=wd: sg[:, a * 512:a * 512 + wd], m, wd)
                e = attp.tile([128, 3 * wd], F16, tag=f"esm{m}", bufs=AVLAG + 1,
                              name="es")
                nc.scalar.activation(
                    e.rearrange("p (a w) -> p a w", w=wd),
                    sg.rearrange("p (a c) -> p a c", c=512)[:, :, 0:wd],
                    AF.Exp, scale=scale)
                # tril mask on the three diagonal 128-wide sub-tiles (DVE;
                # fp16 operands hit the 2x/4x DVE modes, and the AV lag
                # gives ample slack)
                dv = e.rearrange("p (a w) -> p a w", w=wd)[:, :, 0:128]
                nc.vector.tensor_mul(
                    dv, dv,
                    mask6[:, 0:384].rearrange("p (a w) -> p a w", w=128))
                es[m] = e
            # m2 (256) and m3 (128) pack into one bank: one exp, one mask
            sg = sgroup()
            smm(lambda a: sg[:, a * 512:a * 512 + 256], 2, 256)
            smm(lambda a: sg[:, a * 512 + 256:a * 512 + 384], 3, 128)
            e23 = attp.tile([128, 3 * 384], F16, tag="esm23", bufs=AVLAG + 1,
                            name="es")
            nc.scalar.activation(
                e23.rearrange("p (a w) -> p a w", w=384),
                sg.rearrange("p (a c) -> p a c", c=512)[:, :, 0:384],
                AF.Exp, scale=scale)
            # diag sub-tiles: m2's at slab offset 0, m3's at offset 256
            dv = e23.rearrange("p (a j w) -> p a j w", j=3, w=128)[:, :, 0:3:2, :]
            nc.vector.tensor_mul(
                dv, dv,
                mask6.rearrange("p (a j w) -> p a j w", j=2, w=128))
            es[23] = e23
            return es

        def emit_av_recl(qb, h, es, use_sg=False):
            kti, koff = h // 2, (h % 2) * D
            # av fills a [128,512] PSUM bank: rows 0:64 = y, rows 64:128 =
            # the denominator l replicated by the 64 ones-columns of vaug.
            # Drain iterations borrow a freed S-group bank so consecutive
            # AVs' recl->normalize chains overlap.
            av = (sgroup()[0:128, 0:512] if use_sg
                  else avp.tile([128, 512], F32, tag="av", name="av_ps"))
            mv = {0: lambda a: es[0][:, a * 512:(a + 1) * 512],
                  1: lambda a: es[1][:, a * 384:(a + 1) * 384],
                  2: lambda a: es[23][:, a * 384:a * 384 + 256],
                  3: lambda a: es[23][:, a * 384 + 256:a * 384 + 384]}
            first = True
            for m in range(4):
                for a in range(3):
                    nc.tensor.matmul(
                        av[:, m * 128:512],
                        vaug[a * 4 + m][:, h * 2 * D:(h + 1) * 2 * D],
                        mv[m](a),
                        start=first, stop=(m == 3 and a == 2))
                    first = False
            recl = attp.tile([D, 512], F16, tag="recl", bufs=2, name="recl")
            # fp16 reciprocal: 1/l ~ 1e-3..1e-2 is mid fp16 normal range
            # (0.05% rel), far below the fp16-matmul noise floor
            with nc.allow_low_precision(reason="recl fp16"):
                nc.vector.reciprocal(recl[:], av[D:2 * D, :])
            return av, recl

        def emit_norm(qb, h, av, recl):
            kti, koff = h // 2, (h % 2) * D
            nc.vector.tensor_mul(
                yT[kti][koff:koff + D, qb * 512:(qb + 1) * 512],
                av[0:D, :], recl[:])

        # Final q-block: per tile-pair, the kc0/kc1 partial accumulations
        # run first (they need only heads 0-3's normalizes, long done) so
        # PE covers the recl->bcast->norm(17) chain; only the closing kc2
        # matmuls wait on the last normalize.  Both tiles' groups stay open
        # (2 claims, one bank per chunk).
        qb2_pre = {}

        def emit_out_qb2_pre(tpair):
            for mt in tpair:
                ps = sgroup()
                qb2_pre[mt] = ps
                for no, w in ((0, 512), (512, 256)):
                    for kc in (0, 1):
                        nc.tensor.matmul(
                            ps[:, no:no + w],
                            yT[kc][:, mt * 128:(mt + 1) * 128],
                            wp[:, kc * C + no:kc * C + no + w],
                            start=(kc == 0), stop=False)

        def emit_out_proj_qb2():
            for tpair in ((8, 9), (10, 11)):
                osb = iop.tile([128, 2 * C], F32, tag="osb", name="osb")
                if tpair[0] not in qb2_pre:
                    emit_out_qb2_pre(tpair)
                pss = qb2_pre
                for j, mt in enumerate(tpair):
                    ps = pss[mt]
                    for no, w in ((0, 512), (512, 256)):
                        nc.tensor.matmul(
                            ps[:, no:no + w],
                            yT[2][:, mt * 128:(mt + 1) * 128],
                            wp[:, 2 * C + no:2 * C + no + w],
                            start=False, stop=True)
                    if mt == 11:
                        # last tile: split evac+DMA at the chunk boundary
                        for ho, hw_ in ((0, 512), (512, 256)):
                            nc.vector.tensor_copy(
                                osb[:, j * C + ho:j * C + ho + hw_],
                                ps[:, ho:ho + hw_])
                            nc.sync.dma_start(
                                out_d[mt * 128:(mt + 1) * 128, ho:ho + hw_],
                                osb[:, j * C + ho:j * C + ho + hw_])
                    else:
                        nc.vector.tensor_copy(osb[:, j * C:(j + 1) * C],
                                               ps[:, 0:C])
                        nc.sync.dma_start(out_d[mt * 128:(mt + 1) * 128, :],
                                          osb[:, j * C:(j + 1) * C])

        def emit_out_proj(qb):
            if qb == 2:
                emit_out_proj_qb2()
                return
            for pair in range(2):
                osb = iop.tile([128, 2 * C], F32, tag="osb", name="osb")
                for j in range(2):
                    mt = qb * 4 + pair * 2 + j
                    ps = sgroup()
                    for no, w in ((0, 512), (512, 256)):
                        for kc in range(3):
                            nc.tensor.matmul(
                                ps[:, no:no + w],
                                yT[kc][:, mt * 128:(mt + 1) * 128],
                                wp[:, kc * C + no:kc * C + no + w],
                                start=(kc == 0), stop=(kc == 2))
                    nc.vector.tensor_copy(osb[:, j * C:(j + 1) * C],
                                          ps[:, 0:C])
                mt0 = qb * 4 + pair * 2
                nc.sync.dma_start(
                    out_d[mt0 * 128:(mt0 + 2) * 128, :].rearrange(
                        "(j p) c -> p j c", p=128),
                    osb.rearrange("p (j c) -> p j c", c=C))

        iters = [(qb, h) for qb in range(3) for h in range(HG)]
        es_q = [None] * len(iters)
        # out-proj for qb emitted once AV index reaches (last iter of qb)+3,
        # giving the DVE normalize ample slack
        pending = []  # (ready_av_index, qb)

        # Round structure (one AV round j paired with one S round i):
        # AV(j)+recl(j) | S(i) exps/masks | out-proj | norm(j).  The
        # normalize goes LAST in DVE program order so masks/evacs queued
        # on the in-order DVE never delay the next round's S-burst.
        def emit_round(j, i):
            acc = None
            if j is not None:
                acc = emit_av_recl(*iters[j], es_q[j],
                                   use_sg=False)
                es_q[j] = None
            if i is not None and i < 2:
                emit_v_group(i + 2)
            if i is not None:
                es_q[i] = emit_s_burst(*iters[i])
            if j is not None:
                if iters[j][1] == HG - 1:
                    pending.append((j + 2, iters[j][0]))
                while pending and pending[0][0] <= j:
                    emit_out_proj(pending.pop(0)[1])
                emit_norm(*iters[j], acc[0], acc[1])

        for i in range(len(iters)):
            emit_round(i - AVLAG if i >= AVLAG else None, i)
        for j in range(len(iters) - AVLAG, len(iters)):
            emit_round(j, None)
        while pending:
            emit_out_proj(pending.pop(0)[1])

    nc.compile()
    return nc


_NC_CACHE = None


def _get_nc():
    global _NC_CACHE
    if _NC_CACHE is None:
        _NC_CACHE = build_nc()
    return _NC_CACHE


def make_in_maps(inputs):
    x = np.asarray(inputs["x"], dtype=np.float32)
    wq = np.asarray(inputs["Wq"], np.float32)
    wk = np.asarray(inputs["Wk"], np.float32)
    wv = np.asarray(inputs["Wv"], np.float32)
    wp = np.asarray(inputs["Wp"], np.float32)
    bq = np.asarray(inputs["bq"], np.float32)
    bk = np.asarray(inputs["bk"], np.float32)
    bv = np.asarray(inputs["bv"], np.float32)
    bp = np.asarray(inputs["bp"], np.float32)
    triu = np.triu(np.ones((128, 128), dtype=np.float16))
    mask6 = np.tile(triu, (1, 6))
    in_maps = []
    for c in range(NCORES):
        b, g = c // 2, c % 2
        sl = slice(g * CL, (g + 1) * CL)
        cst32 = np.concatenate([
            bq[sl].reshape(3, 128).T,
            bk[sl].reshape(3, 128).T,
            np.tile(bv[sl][None, :], (128, 1)),
        ], axis=1).astype(np.float32)
        m = {
            "xt": np.ascontiguousarray(x[b].T.astype(np.float16)),
            "wqt": np.ascontiguousarray(wq[sl].T.astype(np.float16)),
            "wkt": np.ascontiguousarray(wk[sl].T.astype(np.float16)),
            "wvt": np.ascontiguousarray(wv[sl].T.astype(np.float16)),
            "wpt": np.ascontiguousarray(wp[:, sl].T.astype(np.float16)),
            "cst32": np.ascontiguousarray(cst32),
            "mask6": mask6,
        }
        in_maps.append(m)
    return in_maps


def combine_outputs(results, bp):
    # the output bias is added here: the host already sums the two
    # head-group partials, so folding bp in costs nothing on device
    out = np.empty((B, T, C), dtype=np.float32)
    for b in range(B):
        out[b] = results[2 * b]["out"] + results[2 * b + 1]["out"] + bp
    return out


def kernel(**inputs):
    nc = _get_nc()
    res = run_bass_kernel_spmd(nc, make_in_maps(inputs),
                               core_ids=list(range(NCORES)))
    return combine_outputs(res.results,
                           np.asarray(inputs["bp"], np.float32))
